# revision 1
# baseline (speedup 1.0000x reference)
"""Causal attention (B=4, L=2048, d_model=1024, d_k=d_v=128) on 8 TRN2 NeuronCores.

Sharding (SPMD — one program, per-core data):
  core c -> batch b = c//2, parity par = c%2.
  Core handles q-blocks j = 2k+par for slot k in 0..7 (128 rows each).
  X^T's column blocks are split by parity into two slot-ordered inputs:
  xq (this core's query-parity blocks, which are also half the keys) and
  xo (the other parity's blocks).  Slot k attends key-slots 0..k of EACH
  parity — a uniform instruction stream across cores.  The causal
  boundary is uniform too: the diagonal (triangular) mask always lands on
  q-parity key-slot m == k, while other-parity key-slot m == k is fully
  masked (even cores) or fully valid (odd cores) — fed as mask data.
  Every core projects K/V for all 2048 rows of its batch (KV compute
  duplicated within a pair; no collectives).

Within a core (all matmuls contract on the partition dim):
  - Projections are weight-stationary per 512-column group, accumulating
    8 d_model chunks in PSUM; inputs stream in consumption order and each
    projection group chases its own DMA piece.
  - Scores are computed TRANSPOSED: S^T[key, q] = K^T_blk.T @ Q^T, one
    N<=512 matmul per (parity, key-slot, slot group of 4).  exp() then
    writes A^T straight to SBUF (bf16) — no PE transposes or copies for A.
  - V is augmented with a ones column; Z_aug = A^T.T @ [V | 1] yields the
    softmax denominator in column 128 for free.  Softmax skips the row-max
    subtraction (scores here are bounded ~|12|; exp is safe in f32).
"""

import os
import sys

sys.path.insert(0, "/opt/trn_rl_repo")
sys.path.insert(0, "/opt/trn_rl_repo/concourse")

import ml_dtypes
import numpy as np

import concourse.bass as bass  # noqa: F401
import concourse.mybir as mybir
import concourse.tile as tile
from concourse import bacc
from concourse.bass_utils import run_bass_kernel_spmd
from concourse.masks import make_identity

B, L, DM, DK, DV = 4, 2048, 1024, 128, 128
NB = L // 128   # 16 key blocks per batch
SLOTS = 8       # q-blocks per core
NCH = DM // 128  # 8 d_model chunks
SCALE = float(DK) ** -0.5
MASKVAL = -1e9

COMPUTE = os.environ.get("ATTN_COMPUTE", "bf16")  # "bf16" | "f32"

F32 = mybir.dt.float32


def _cdt():
    return mybir.dt.bfloat16 if COMPUTE == "bf16" else mybir.dt.float32


def _np_cdt():
    return ml_dtypes.bfloat16 if COMPUTE == "bf16" else np.float32


def build_nc():
    cdt = _cdt()
    nc = bacc.Bacc()

    # X^T columns split by parity, each slot-ordered: xq = this core's
    # query-parity blocks (also half the keys), xo = other-parity blocks
    xq_ext = nc.declare_dram_parameter("xq", [DM, SLOTS * 128], cdt, isOutput=False)
    xo_ext = nc.declare_dram_parameter("xo", [DM, SLOTS * 128], cdt, isOutput=False)
    # weights pre-arranged on host to the SBUF chunk layout
    # [p, c*128+d] = W[c*128+p, d] so the DMA is fully contiguous
    wq_ext = nc.declare_dram_parameter("wq", [128, DM], cdt, isOutput=False)
    wk_ext = nc.declare_dram_parameter("wk", [128, DM], cdt, isOutput=False)
    wv_ext = nc.declare_dram_parameter("wv", [128, DM], cdt, isOutput=False)
    # transposed boundary masks: [key 128, 2*128 q] — col block 0 applied at
    # key block 2k, col block 1 at key block 2k+1 (for slot k)
    mask_ext = nc.declare_dram_parameter("maskT", [128, 256], F32, isOutput=False)
    out_ext = nc.declare_dram_parameter("out", [SLOTS * 128, DV], F32, isOutput=True)

    with tile.TileContext(nc) as tc:
        with (
            tc.tile_pool(name="persist", bufs=1) as persist,
            tc.tile_pool(name="mm_ps", bufs=6, space="PSUM") as mm_ps,
            tc.tile_pool(name="z_ps", bufs=2, space="PSUM") as z_ps,
            tc.tile_pool(name="work", bufs=6) as work,
        ):
            # ---- constants / inputs ----
            ident = persist.tile([128, 128], cdt, tag="ident")
            make_identity(nc, ident)

            w_sb = {}

            def load_w(name, ext):
                t = persist.tile([128, NCH, 128], cdt, tag=name, name=name)
                nc.sync.dma_start(
                    out=t[:], in_=ext.rearrange("p (c d) -> p c d", d=128)
                )
                w_sb[name] = t

            # Every DMA gets its own tile sized to exactly one consumer's
            # need (dependency tracking is DMA-granular): 512-column pieces
            # spanning all 8 d_model chunks; projection group g chases
            # piece g.
            xq_r = xq_ext.rearrange("(c p) l -> p c l", p=128)
            xo_r = xo_ext.rearrange("(c p) l -> p c l", p=128)
            # single queue => ring order == issue order == consumption order
            def piece(r, lo, w, nm):
                t = persist.tile([128, NCH, w], cdt, tag=nm, name=nm)
                nc.sync.dma_start(out=t[:], in_=r[:, :, lo:lo + w])
                return t

            load_w("wq", wq_ext)
            # first 512 columns split in two so the PE can start after 0.5MB
            xq_a = piece(xq_r, 0, 256, "xqa")
            xq_b = piece(xq_r, 256, 256, "xqb")
            mask_sb = persist.tile([128, 256], F32, tag="mask")
            nc.sync.dma_start(out=mask_sb[:], in_=mask_ext[:])
            load_w("wk", wk_ext)
            load_w("wv", wv_ext)
            xq_c = piece(xq_r, 512, 512, "xqc")
            xo_a = piece(xo_r, 0, 512, "xoa")
            xo_b = piece(xo_r, 512, 512, "xob")
            # per projection group: list of (rhs-piece, psum column offset)
            xq_p = [[(xq_a, 0), (xq_b, 256)], [(xq_c, 0)]]
            xo_p = [[(xo_a, 0)], [(xo_b, 0)]]

            # ---- per-(parity s, group) tiles; s=0 query-parity, s=1 other
            qt = [persist.tile([128, 512], cdt, tag=f"qt{g}", name=f"qt{g}")
                  for g in range(2)]
            kt = {(sp, g): persist.tile([128, 512], cdt, tag=f"kt{sp}{g}",
                                        name=f"kt{sp}{g}")
                  for sp in range(2) for g in range(2)}
            vt = {(sp, g): persist.tile([128, 512], cdt, tag=f"vt{sp}{g}",
                                        name=f"vt{sp}{g}")
                  for sp in range(2) for g in range(2)}
            v_aug = {}
            for sp in range(2):
                for m in range(SLOTS):
                    t = persist.tile([128, DV + 1], cdt, tag=f"va{sp}{m}",
                                     name=f"va{sp}{m}")
                    nc.vector.memset(t[:, DV:DV + 1], 1.0)
                    v_aug[(sp, m)] = t
            at = {}
            for sp in range(2):
                for m in range(SLOTS):
                    for g in range(2):
                        if m <= 4 * g + 3:
                            at[(sp, m, g)] = persist.tile(
                                [128, 512], cdt, tag=f"at{sp}_{m}_{g}",
                                name=f"at{sp}_{m}_{g}")

            def proj(name, src, dst, scale, gs):
                w = w_sb[name]
                for g in gs:
                    for pi, (t, off) in enumerate(src[g]):
                        wd = t.shape[-1]
                        ps = mm_ps.tile([128, wd], F32, tag="mm",
                                        name=f"pj{g}_{pi}")
                        for c in range(NCH):
                            nc.tensor.matmul(
                                ps[:],
                                w[:, c, :],
                                t[:, c, :],
                                start=(c == 0),
                                stop=(c == NCH - 1),
                            )
                        dslice = dst[g][:, off:off + wd]
                        if scale is not None:
                            nc.scalar.activation(
                                dslice, ps[:],
                                mybir.ActivationFunctionType.Copy,
                                bias=0.0, scale=scale,
                            )
                        elif name == "wv":
                            # keep V^T copies off the Scalar engine (it owns
                            # the exps the V-transposes otherwise wait behind)
                            nc.vector.tensor_copy(dslice, ps[:])
                        else:
                            nc.scalar.copy(dslice, ps[:])

            # emission in stream-arrival order; the Tile scheduler
            # dispatches by readiness + this priority
            def vt_blocks(sp, ms):
                for m in ms:
                    vps = mm_ps.tile([128, 128], cdt, tag="mm", name="vps")
                    nc.tensor.transpose(
                        vps[:],
                        vt[(sp, m // 4)][:, (m % 4) * 128:(m % 4 + 1) * 128],
                        ident[:],
                    )
                    dst = v_aug[(sp, m)][:, 0:DV]
                    nc.vector.tensor_copy(dst, vps[:])

            def scores(sp, ms):
                # S^T for key-slot m of parity sp, covered by q-slots k >= m
                for m in ms:
                    for g in range(2):
                        lo = max(m, 4 * g)
                        if lo > 4 * g + 3:
                            continue
                        a = lo - 4 * g
                        st = mm_ps.tile([128, 512], F32, tag="mm")
                        nc.tensor.matmul(
                            st[:, a * 128:512],
                            kt[(sp, m // 4)][:, (m % 4) * 128:(m % 4 + 1) * 128],
                            qt[g][:, a * 128:512],
                            start=True, stop=True,
                            skip_group_check=True,
                        )
                        if 4 * g <= m <= 4 * g + 3:
                            # causal boundary: q-parity slot m gets the
                            # triangle, other-parity slot m is all-or-nothing
                            # by core parity (mask data)
                            qoff = (m - 4 * g) * 128
                            nc.vector.tensor_add(
                                st[:, qoff:qoff + 128],
                                st[:, qoff:qoff + 128],
                                mask_sb[:, sp * 128:(sp + 1) * 128],
                            )
                        nc.scalar.activation(
                            at[(sp, m, g)][:, a * 128:512],
                            st[:, a * 128:512],
                            mybir.ActivationFunctionType.Exp,
                            bias=0.0, scale=1.0,
                        )

            def av(ks):
                for k in ks:
                    g, q = k // 4, (k % 4) * 128
                    zp = z_ps.tile([128, DV + 1], F32, tag="z")
                    for m in range(k + 1):
                        for sp in range(2):
                            nc.tensor.matmul(
                                zp[:],
                                at[(sp, m, g)][:, q:q + 128],
                                v_aug[(sp, m)][:],
                                start=(m == 0 and sp == 0),
                                stop=(m == k and sp == 1),
                            )
                    rcp = work.tile([128, 1], F32, tag="rcp")
                    nc.vector.reciprocal(rcp[:], zp[:, DV:DV + 1])
                    z_sb = work.tile([128, DV], F32, tag="zout")
                    nc.vector.tensor_scalar_mul(z_sb[:], zp[:, 0:DV], rcp[:])
                    nc.scalar.dma_start(
                        out=out_ext[k * 128:(k + 1) * 128, :], in_=z_sb[:]
                    )

            proj("wq", xq_p, qt, SCALE, [0])
            proj("wk", xq_p, [kt[(0, 0)], kt[(0, 1)]], None, [0])
            proj("wv", xq_p, [vt[(0, 0)], vt[(0, 1)]], None, [0])
            proj("wq", xq_p, qt, SCALE, [1])
            vt_blocks(0, range(0, 4))
            scores(0, range(0, 4))
            proj("wk", xq_p, [kt[(0, 0)], kt[(0, 1)]], None, [1])
            proj("wv", xq_p, [vt[(0, 0)], vt[(0, 1)]], None, [1])
            vt_blocks(0, range(4, 8))
            scores(0, range(4, 8))
            proj("wk", xo_p, [kt[(1, 0)], kt[(1, 1)]], None, [0])
            proj("wv", xo_p, [vt[(1, 0)], vt[(1, 1)]], None, [0])
            vt_blocks(1, range(0, 4))
            scores(1, range(0, 4))
            av(range(0, 4))
            proj("wk", xo_p, [kt[(1, 0)], kt[(1, 1)]], None, [1])
            proj("wv", xo_p, [vt[(1, 0)], vt[(1, 1)]], None, [1])
            vt_blocks(1, range(4, 8))
            scores(1, range(4, 8))
            av(range(4, 8))

    nc.finalize()
    return nc


_NC = None


def _get_nc():
    global _NC
    if _NC is None:
        _NC = build_nc()
    return _NC


def _make_masks():
    p = np.arange(128)[:, None]   # key (partition)
    q = np.arange(128)[None, :]   # query (free)
    triT = np.where(p <= q, 0.0, MASKVAL).astype(np.float32)
    full = np.full((128, 128), MASKVAL, np.float32)
    zero = np.zeros((128, 128), np.float32)
    # col block 0: q-parity key-slot m == k (diagonal, both parities);
    # col block 1: other-parity key-slot m == k (all-masked on even cores,
    # all-valid on odd cores)
    mask_even = np.concatenate([triT, full], axis=1)
    mask_odd = np.concatenate([triT, zero], axis=1)
    return mask_even, mask_odd


def kernel(X, W_Q, W_K, W_V):
    X = np.asarray(X, np.float32)
    W_Q = np.asarray(W_Q, np.float32)
    W_K = np.asarray(W_K, np.float32)
    W_V = np.asarray(W_V, np.float32)

    nc = _get_nc()
    npdt = _np_cdt()
    mask_even, mask_odd = _make_masks()

    def warr(W):
        return np.ascontiguousarray(
            W.astype(npdt).reshape(NCH, 128, DK).transpose(1, 0, 2)
            .reshape(128, NCH * DK)
        )

    wq = warr(W_Q)
    wk = warr(W_K)
    wv = warr(W_V)

    in_maps = []
    for c in range(8):
        b, par = c // 2, c % 2
        xt_np = np.ascontiguousarray(X[b].T).astype(npdt)
        qcols = np.concatenate(
            [np.arange((2 * k + par) * 128, (2 * k + par + 1) * 128)
             for k in range(SLOTS)]
        )
        ocols = np.concatenate(
            [np.arange((2 * k + 1 - par) * 128, (2 * k + 2 - par) * 128)
             for k in range(SLOTS)]
        )
        in_maps.append({
            "xq": np.ascontiguousarray(xt_np[:, qcols]),
            "xo": np.ascontiguousarray(xt_np[:, ocols]),
            "wq": wq, "wk": wk, "wv": wv,
            "maskT": mask_odd if par else mask_even,
        })

    res = run_bass_kernel_spmd(nc, in_maps, list(range(8)))

    Z = np.zeros((B, L, DV), np.float32)
    for c in range(8):
        b, par = c // 2, c % 2
        o = res.results[c]["out"]
        for k in range(SLOTS):
            j = 2 * k + par
            Z[b, j * 128:(j + 1) * 128, :] = o[k * 128:(k + 1) * 128, :]
    return Z



# revision 7
# speedup vs baseline: 1.0248x; 1.0248x over previous
"""Causal attention (B=4, L=2048, d_model=1024, d_k=d_v=128) on 8 TRN2 NeuronCores.

Sharding (SPMD — one program, per-core data):
  core c -> batch b = c//2, parity par = c%2.
  Core handles q-blocks j = 2k+par for slot k in 0..7 (128 rows each).
  X^T's column blocks are split by parity into two slot-ordered inputs:
  xq (this core's query-parity blocks, which are also half the keys) and
  xo (the other parity's blocks).  Slot k attends key-slots 0..k of EACH
  parity — a uniform instruction stream across cores.  The causal
  boundary is uniform too: the diagonal (triangular) mask always lands on
  q-parity key-slot m == k, while other-parity key-slot m == k is fully
  masked (even cores) or fully valid (odd cores) — fed as mask data.
  Every core projects K/V for all 2048 rows of its batch (KV compute
  duplicated within a pair; no collectives — an 8-core AllGather's
  ~5-10us floor exceeds the 6.8us of duplicated projection work).

Within a core (all matmuls contract on the partition dim):
  - A burst of throwaway matmuls on a memset tile runs first so the
    PE_HAM clock gate reaches 2.4 GHz before real data arrives.
  - Each input DMA piece is packed host-side to [128, NCH*w] so every
    partition is one contiguous DRAM run (128 descriptors, fast HWDGE
    issue).  Stream order: wq, xq:128c, xq:384c, wk, wv, xq:512c,
    xo:512c, xo:512c — the first matmul needs only wq + 0.25 MB of X.
  - Projections are weight-stationary per column group, accumulating 8
    d_model chunks in PSUM; emission follows stream-arrival order so the
    in-order Tensor queue never stalls on a later piece.
  - Scores are computed TRANSPOSED: S^T[key, q] = K^T_blk.T @ Q^T; the
    1/sqrt(d_k) scale is folded into W_Q host-side.  exp() (Scalar — the
    only engine with ACT) writes A^T straight to SBUF in bf16.  Scalar
    does nothing else; PSUM->SBUF copies go to Vector/GpSimd and the
    boundary-mask adds to GpSimd.
  - V is augmented with a ones column; Z_aug = A^T.T @ [V | 1] yields the
    softmax denominator in column 128 for free.  Softmax skips the
    row-max subtraction (scores here are bounded ~|12|; exp is safe).
"""

import os
import sys

sys.path.insert(0, "/opt/trn_rl_repo")
sys.path.insert(0, "/opt/trn_rl_repo/concourse")

import ml_dtypes
import numpy as np

import concourse.bass as bass  # noqa: F401
import concourse.mybir as mybir
import concourse.tile as tile
from concourse import bacc
from concourse.bass_utils import run_bass_kernel_spmd
from concourse.masks import make_identity

B, L, DM, DK, DV = 4, 2048, 1024, 128, 128
NB = L // 128   # 16 key blocks per batch
SLOTS = 8       # q-blocks per core
NCH = DM // 128  # 8 d_model chunks
SCALE = float(DK) ** -0.5
MASKVAL = -1e9

COMPUTE = os.environ.get("ATTN_COMPUTE", "bf16")  # "bf16" | "f32"
N_WARM = int(os.environ.get("ATTN_WARM", "8"))

F32 = mybir.dt.float32

# (name, source, column range) of each X piece in stream order; xq/xo are
# the parity-split X^T halves, 1024 columns each
PIECES = [
    ("xq0", "q", 0, 128),
    ("xq1", "q", 128, 384),
    ("xq2", "q", 512, 512),
    ("xo0", "o", 0, 512),
    ("xo1", "o", 512, 512),
]


def _cdt():
    return mybir.dt.bfloat16 if COMPUTE == "bf16" else mybir.dt.float32


def _np_cdt():
    return ml_dtypes.bfloat16 if COMPUTE == "bf16" else np.float32


def build_nc():
    cdt = _cdt()
    nc = bacc.Bacc()

    # weights pre-arranged on host to the SBUF chunk layout
    # [p, c*128+d] = W[c*128+p, d] so the DMA is fully contiguous
    wq_ext = nc.declare_dram_parameter("wq", [128, DM], cdt, isOutput=False)
    wk_ext = nc.declare_dram_parameter("wk", [128, DM], cdt, isOutput=False)
    wv_ext = nc.declare_dram_parameter("wv", [128, DM], cdt, isOutput=False)
    # X pieces, each packed host-side as [p, c*w+l] (one contiguous DRAM
    # run per partition)
    piece_ext = {
        nm: nc.declare_dram_parameter(nm, [128, NCH * w], cdt, isOutput=False)
        for nm, _, _, w in PIECES
    }
    # transposed multiplicative boundary masks (0/1): [key 128, 2*128 q] —
    # col block 0 applied at key block 2k, col block 1 at key block 2k+1
    mask_ext = nc.declare_dram_parameter("maskT", [128, 256], cdt, isOutput=False)
    out_ext = nc.declare_dram_parameter("out", [SLOTS * 128, DV], F32, isOutput=True)

    with tile.TileContext(nc) as tc:
        with (
            tc.tile_pool(name="persist", bufs=1) as persist,
            tc.tile_pool(name="mm_ps", bufs=6, space="PSUM") as mm_ps,
            tc.tile_pool(name="z_ps", bufs=2, space="PSUM") as z_ps,
            tc.tile_pool(name="work", bufs=6) as work,
        ):
            # ---- PE warm-up: matmuls on a zeroed tile so the HAM clock
            # gate flips to full rate before the first real matmul ----
            warm_t = persist.tile([128, 512], cdt, tag="warmt")
            nc.vector.memset(warm_t[:], 0.0)
            for i in range(N_WARM):
                wp = mm_ps.tile([128, 512], F32, tag="mm", name=f"warm{i}")
                nc.tensor.matmul(wp[:], warm_t[:, 0:128], warm_t[:],
                                 start=True, stop=True)

            # ---- constants / inputs ----
            ident = persist.tile([128, 128], cdt, tag="ident")
            make_identity(nc, ident)

            w_sb = {}

            def load_w(name, ext):
                t = persist.tile([128, NCH, 128], cdt, tag=name, name=name)
                nc.sync.dma_start(
                    out=t[:], in_=ext.rearrange("p (c d) -> p c d", d=128)
                )
                w_sb[name] = t

            pc_sb = {}

            def load_piece(nm, w):
                t = persist.tile([128, NCH, w], cdt, tag=nm, name=nm)
                nc.sync.dma_start(
                    out=t[:],
                    in_=piece_ext[nm].rearrange("p (c l) -> p c l", l=w),
                )
                pc_sb[nm] = t

            # single queue => ring order == issue order == consumption
            # order; first matmul needs only wq + xq0
            load_w("wq", wq_ext)
            load_piece("xq0", 128)
            load_piece("xq1", 384)
            load_w("wk", wk_ext)
            load_w("wv", wv_ext)
            load_piece("xq2", 512)
            load_piece("xo0", 512)
            load_piece("xo1", 512)
            mask_sb = persist.tile([128, 256], cdt, tag="mask")
            nc.gpsimd.dma_start(out=mask_sb[:], in_=mask_ext[:])

            # per (source, group): list of (piece tile, psum col offset)
            xq_p = [[(pc_sb["xq0"], 0), (pc_sb["xq1"], 128)],
                    [(pc_sb["xq2"], 0)]]
            xo_p = [[(pc_sb["xo0"], 0)], [(pc_sb["xo1"], 0)]]

            # ---- per-(parity s, group) tiles; s=0 query-parity, s=1 other
            qt = [persist.tile([128, 512], cdt, tag=f"qt{g}", name=f"qt{g}")
                  for g in range(2)]
            kt = {(sp, g): persist.tile([128, 512], cdt, tag=f"kt{sp}{g}",
                                        name=f"kt{sp}{g}")
                  for sp in range(2) for g in range(2)}
            vt = {(sp, g): persist.tile([128, 512], cdt, tag=f"vt{sp}{g}",
                                        name=f"vt{sp}{g}")
                  for sp in range(2) for g in range(2)}
            v_aug = {}
            for sp in range(2):
                for m in range(SLOTS):
                    t = persist.tile([128, DV + 1], cdt, tag=f"va{sp}{m}",
                                     name=f"va{sp}{m}")
                    nc.vector.memset(t[:, DV:DV + 1], 1.0)
                    v_aug[(sp, m)] = t
            at = {}
            for sp in range(2):
                for m in range(SLOTS):
                    for g in range(2):
                        if m <= 4 * g + 3:
                            at[(sp, m, g)] = persist.tile(
                                [128, 512], cdt, tag=f"at{sp}_{m}_{g}",
                                name=f"at{sp}_{m}_{g}")

            def proj(name, src, dst, gs, copy_eng):
                w = w_sb[name]
                for g in gs:
                    for pi, (t, off) in enumerate(src[g]):
                        wd = t.shape[-1]
                        ps = mm_ps.tile([128, wd], F32, tag="mm",
                                        name=f"pj{g}_{pi}")
                        for c in range(NCH):
                            nc.tensor.matmul(
                                ps[:],
                                w[:, c, :],
                                t[:, c, :],
                                start=(c == 0),
                                stop=(c == NCH - 1),
                            )
                        copy_eng(dst[g][:, off:off + wd], ps[:])

            # emission in stream-arrival order; each engine's queue
            # executes in order, so never emit ahead of the data
            def vt_blocks(sp, ms):
                for m in ms:
                    vps = mm_ps.tile([128, 128], cdt, tag="mm", name="vps")
                    nc.tensor.transpose(
                        vps[:],
                        vt[(sp, m // 4)][:, (m % 4) * 128:(m % 4 + 1) * 128],
                        ident[:],
                    )
                    dst = v_aug[(sp, m)][:, 0:DV]
                    nc.vector.tensor_copy(dst, vps[:])

            def scores(sp, mgs):
                # S^T for key-slot m of parity sp over q-group g
                for m, g in mgs:
                    lo = max(m, 4 * g)
                    if lo > 4 * g + 3:
                        continue
                    a = lo - 4 * g
                    st = mm_ps.tile([128, 512], F32, tag="mm")
                    nc.tensor.matmul(
                        st[:, a * 128:512],
                        kt[(sp, m // 4)][:, (m % 4) * 128:(m % 4 + 1) * 128],
                        qt[g][:, a * 128:512],
                        start=True, stop=True,
                        skip_group_check=True,
                    )
                    nc.scalar.activation(
                        at[(sp, m, g)][:, a * 128:512],
                        st[:, a * 128:512],
                        mybir.ActivationFunctionType.Exp,
                        bias=0.0, scale=1.0,
                    )
                    if 4 * g <= m <= 4 * g + 3:
                        # causal boundary applied POST-exp as a 0/1
                        # multiply (GpSimd can't touch PSUM): q-parity
                        # slot m gets the triangle, other-parity slot m is
                        # all-or-nothing by core parity (mask data).
                        # |score| <= ~12 so unmasked exp cannot overflow.
                        qoff = (m - 4 * g) * 128
                        asl = at[(sp, m, g)][:, qoff:qoff + 128]
                        nc.gpsimd.tensor_mul(
                            asl, asl, mask_sb[:, sp * 128:(sp + 1) * 128],
                        )

            def av(ks):
                for k in ks:
                    g, q = k // 4, (k % 4) * 128
                    zp = z_ps.tile([128, DV + 1], F32, tag="z")
                    for m in range(k + 1):
                        for sp in range(2):
                            nc.tensor.matmul(
                                zp[:],
                                at[(sp, m, g)][:, q:q + 128],
                                v_aug[(sp, m)][:],
                                start=(m == 0 and sp == 0),
                                stop=(m == k and sp == 1),
                            )
                    rcp = work.tile([128, 1], F32, tag="rcp")
                    nc.vector.reciprocal(rcp[:], zp[:, DV:DV + 1])
                    z_sb = work.tile([128, DV], F32, tag="zout")
                    nc.vector.tensor_scalar_mul(z_sb[:], zp[:, 0:DV], rcp[:])
                    nc.sync.dma_start(
                        out=out_ext[k * 128:(k + 1) * 128, :], in_=z_sb[:]
                    )

            vcopy = nc.vector.tensor_copy
            scopy = nc.scalar.copy

            # -- xq0+xq1 (parity-0 key blocks 0..3) + all weights --
            proj("wq", xq_p, qt, [0], vcopy)
            proj("wk", xq_p, [kt[(0, 0)], kt[(0, 1)]], [0], scopy)
            proj("wv", xq_p, [vt[(0, 0)], vt[(0, 1)]], [0], vcopy)
            vt_blocks(0, range(0, 4))
            scores(0, [(m, 0) for m in range(0, 4)])
            # -- xq2 (parity-0 key blocks 4..7) --
            proj("wq", xq_p, qt, [1], vcopy)
            proj("wk", xq_p, [kt[(0, 0)], kt[(0, 1)]], [1], scopy)
            proj("wv", xq_p, [vt[(0, 0)], vt[(0, 1)]], [1], vcopy)
            scores(0, [(m, 1) for m in range(0, 4)])
            vt_blocks(0, range(4, 8))
            scores(0, [(m, 1) for m in range(4, 8)])
            # -- xo0 (parity-1 key blocks 0..3) --
            proj("wk", xo_p, [kt[(1, 0)], kt[(1, 1)]], [0], scopy)
            proj("wv", xo_p, [vt[(1, 0)], vt[(1, 1)]], [0], vcopy)
            vt_blocks(1, range(0, 4))
            scores(1, [(m, g) for m in range(0, 4) for g in range(2)])
            av(range(0, 4))
            # -- xo1 (parity-1 key blocks 4..7) --
            proj("wk", xo_p, [kt[(1, 0)], kt[(1, 1)]], [1], scopy)
            proj("wv", xo_p, [vt[(1, 0)], vt[(1, 1)]], [1], vcopy)
            vt_blocks(1, range(4, 8))
            scores(1, [(m, 1) for m in range(4, 8)])
            av(range(4, 8))

    nc.finalize()
    return nc


_NC = None


def _get_nc():
    global _NC
    if _NC is None:
        _NC = build_nc()
    return _NC


def _make_masks():
    npdt = _np_cdt()
    p = np.arange(128)[:, None]   # key (partition)
    q = np.arange(128)[None, :]   # query (free)
    # multiplicative 0/1 masks applied to exp(scores)
    triT = (p <= q).astype(npdt)
    dead = np.zeros((128, 128), npdt)
    live = np.ones((128, 128), npdt)
    # col block 0: q-parity key-slot m == k (diagonal, both parities);
    # col block 1: other-parity key-slot m == k (all-masked on even cores,
    # all-valid on odd cores)
    mask_even = np.concatenate([triT, dead], axis=1)
    mask_odd = np.concatenate([triT, live], axis=1)
    return mask_even, mask_odd


def kernel(X, W_Q, W_K, W_V):
    X = np.asarray(X, np.float32)
    W_Q = np.asarray(W_Q, np.float32)
    W_K = np.asarray(W_K, np.float32)
    W_V = np.asarray(W_V, np.float32)

    nc = _get_nc()
    npdt = _np_cdt()
    mask_even, mask_odd = _make_masks()

    def warr(W):
        return np.ascontiguousarray(
            W.astype(npdt).reshape(NCH, 128, DK).transpose(1, 0, 2)
            .reshape(128, NCH * DK)
        )

    wq = warr(W_Q * SCALE)   # fold the 1/sqrt(d_k) into W_Q
    wk = warr(W_K)
    wv = warr(W_V)

    in_maps = []
    for c in range(8):
        b, par = c // 2, c % 2
        # [c, p, l] chunks of X^T for this batch, parity-split and
        # slot-ordered along l
        xt = np.ascontiguousarray(X[b].T).astype(npdt).reshape(NCH, 128, L)
        qcols = np.concatenate(
            [np.arange((2 * k + par) * 128, (2 * k + par + 1) * 128)
             for k in range(SLOTS)]
        )
        ocols = np.concatenate(
            [np.arange((2 * k + 1 - par) * 128, (2 * k + 2 - par) * 128)
             for k in range(SLOTS)]
        )
        src = {"q": xt[:, :, qcols], "o": xt[:, :, ocols]}
        m = {"wq": wq, "wk": wk, "wv": wv,
             "maskT": mask_odd if par else mask_even}
        for nm, s, lo, w in PIECES:
            # pack as [p, c*w+l]: one contiguous DRAM run per partition
            m[nm] = np.ascontiguousarray(
                src[s][:, :, lo:lo + w].transpose(1, 0, 2).reshape(128, NCH * w)
            )
        in_maps.append(m)

    res = run_bass_kernel_spmd(nc, in_maps, list(range(8)))

    Z = np.zeros((B, L, DV), np.float32)
    for c in range(8):
        b, par = c // 2, c % 2
        o = res.results[c]["out"]
        for k in range(SLOTS):
            j = 2 * k + par
            Z[b, j * 128:(j + 1) * 128, :] = o[k * 128:(k + 1) * 128, :]
    return Z


# revision 14
# speedup vs baseline: 1.0551x; 1.0296x over previous
"""Causal attention (B=4, L=2048, d_model=1024, d_k=d_v=128) on 8 TRN2 NeuronCores.

Sharding (SPMD — one program, per-core data):
  core c -> batch b = c//2, parity par = c%2.
  Core handles q-blocks j = 2k+par for slot k in 0..7 (128 rows each).
  X^T's column blocks are split by parity into two slot-ordered inputs:
  xq (this core's query-parity blocks, which are also half the keys) and
  xo (the other parity's blocks).  Slot k attends key-slots 0..k of EACH
  parity — a uniform instruction stream across cores.  The causal
  boundary is uniform too: the diagonal (triangular) mask always lands on
  q-parity key-slot m == k, while other-parity key-slot m == k is fully
  masked (even cores) or fully valid (odd cores) — fed as mask data.
  Every core projects K/V for all 2048 rows of its batch (KV compute
  duplicated within a pair; no collectives — an 8-core AllGather's
  ~5-10us floor exceeds the 6.8us of duplicated projection work).

Within a core (all matmuls contract on the partition dim):
  - A burst of throwaway matmuls on a memset tile runs first so the
    PE_HAM clock gate reaches 2.4 GHz before real data arrives.
  - Each input DMA piece is packed host-side to [128, NCH*w] so every
    partition is one contiguous DRAM run (128 descriptors, fast HWDGE
    issue).  Stream order: wq, xq:128c, xq:384c, wk, wv, xq:512c,
    xo:512c, xo:512c — the first matmul needs only wq + 0.25 MB of X.
  - Projections are weight-stationary per column group, accumulating 8
    d_model chunks in PSUM; emission follows stream-arrival order so the
    in-order Tensor queue never stalls on a later piece.
  - Scores are computed TRANSPOSED: S^T[key, q] = K^T_blk.T @ Q^T; the
    1/sqrt(d_k) scale is folded into W_Q host-side.  exp() (Scalar — the
    only engine with ACT) writes A^T straight to SBUF in bf16.  Scalar
    does nothing else; PSUM->SBUF copies go to Vector/GpSimd and the
    boundary-mask adds to GpSimd.
  - V is augmented with a ones column; Z_aug = A^T.T @ [V | 1] yields the
    softmax denominator in column 128 for free.  Softmax skips the
    row-max subtraction (scores here are bounded ~|12|; exp is safe).
"""

import os
import sys

sys.path.insert(0, "/opt/trn_rl_repo")
sys.path.insert(0, "/opt/trn_rl_repo/concourse")

import ml_dtypes
import numpy as np

import concourse.bass as bass  # noqa: F401
import concourse.mybir as mybir
import concourse.tile as tile
from concourse import bacc
from concourse.bass_utils import run_bass_kernel_spmd
from concourse.masks import make_identity

B, L, DM, DK, DV = 4, 2048, 1024, 128, 128
NB = L // 128   # 16 key blocks per batch
SLOTS = 8       # q-blocks per core
NCH = DM // 128  # 8 d_model chunks
SCALE = float(DK) ** -0.5
MASKVAL = -1e9

COMPUTE = os.environ.get("ATTN_COMPUTE", "bf16")  # "bf16" | "f32"
N_WARM = int(os.environ.get("ATTN_WARM", "8"))

F32 = mybir.dt.float32

# (name, source, column range) of each X piece in stream order; xq/xo are
# the parity-split X^T halves, 1024 columns each.  Fine early pieces so
# the first projections start as soon as possible; the DMA ring is
# ordered wq, xq0, wk, xq1, wv, xq2, xq3, xo0, xo1 (weights interleaved
# so K/V projections are never weight-blocked).
PIECES = [
    ("xq0", "q", 0, 128),
    ("xq1", "q", 128, 128),
    ("xq2", "q", 256, 256),
    ("xq3", "q", 512, 512),
    ("xo0", "o", 0, 512),
    ("xo1", "o", 512, 512),
]


def _cdt():
    return mybir.dt.bfloat16 if COMPUTE == "bf16" else mybir.dt.float32


def _np_cdt():
    return ml_dtypes.bfloat16 if COMPUTE == "bf16" else np.float32


def build_nc():
    cdt = _cdt()
    nc = bacc.Bacc()

    # weights pre-arranged on host to the SBUF chunk layout
    # [p, c*128+d] = W[c*128+p, d] so the DMA is fully contiguous
    wq_ext = nc.declare_dram_parameter("wq", [128, DM], cdt, isOutput=False)
    wk_ext = nc.declare_dram_parameter("wk", [128, DM], cdt, isOutput=False)
    wv_ext = nc.declare_dram_parameter("wv", [128, DM], cdt, isOutput=False)
    # X pieces, each packed host-side as [p, c*w+l] (one contiguous DRAM
    # run per partition)
    piece_ext = {
        nm: nc.declare_dram_parameter(nm, [128, NCH * w], cdt, isOutput=False)
        for nm, _, _, w in PIECES
    }
    # transposed multiplicative boundary masks (0/1): [key 128, 2*128 q] —
    # col block 0 applied at key block 2k, col block 1 at key block 2k+1
    mask_ext = nc.declare_dram_parameter("maskT", [128, 256], cdt, isOutput=False)
    out_ext = nc.declare_dram_parameter("out", [SLOTS * 128, DV], F32, isOutput=True)

    with tile.TileContext(nc) as tc:
        with (
            tc.tile_pool(name="persist", bufs=1) as persist,
            tc.tile_pool(name="mm_ps", bufs=6, space="PSUM") as mm_ps,
            tc.tile_pool(name="z_ps", bufs=2, space="PSUM") as z_ps,
            tc.tile_pool(name="work", bufs=6) as work,
        ):
            # ---- PE warm-up: matmuls on a zeroed tile so the HAM clock
            # gate flips to full rate before the first real matmul; also
            # sprinkled into early DMA-wait bubbles to keep it warm ----
            warm_t = persist.tile([128, 512], cdt, tag="warmt")
            nc.vector.memset(warm_t[:], 0.0)

            def warm(n):
                # z_ps is free until the AV phase, long after warm-ups
                for _ in range(n):
                    wp = z_ps.tile([128, 512], F32, tag="z", name="warm")
                    nc.tensor.matmul(wp[:], warm_t[:, 0:128], warm_t[:],
                                     start=True, stop=True)

            warm(N_WARM)

            # ---- constants / inputs ----
            ident = persist.tile([128, 128], cdt, tag="ident")
            make_identity(nc, ident)

            w_sb = {}

            def load_w(name, ext):
                t = persist.tile([128, NCH, 128], cdt, tag=name, name=name)
                nc.sync.dma_start(
                    out=t[:], in_=ext.rearrange("p (c d) -> p c d", d=128)
                )
                w_sb[name] = t

            pc_sb = {}

            def load_piece(nm, w):
                t = persist.tile([128, NCH, w], cdt, tag=nm, name=nm)
                nc.sync.dma_start(
                    out=t[:],
                    in_=piece_ext[nm].rearrange("p (c l) -> p c l", l=w),
                )
                pc_sb[nm] = t

            # single queue => ring order == issue order == consumption
            # order; first matmul needs only wq + xq0, and the weights are
            # interleaved so K/V projections are never weight-blocked
            load_w("wq", wq_ext)
            load_piece("xq0", 128)
            load_w("wk", wk_ext)
            load_piece("xq1", 128)
            load_w("wv", wv_ext)
            load_piece("xq2", 256)
            load_piece("xq3", 512)
            load_piece("xo0", 512)
            load_piece("xo1", 512)
            mask_sb = persist.tile([128, 256], cdt, tag="mask")
            nc.gpsimd.dma_start(out=mask_sb[:], in_=mask_ext[:])

            # per (source, group): list of (piece tile, psum col offset)
            xq_p = [[(pc_sb["xq0"], 0), (pc_sb["xq1"], 128),
                     (pc_sb["xq2"], 256)],
                    [(pc_sb["xq3"], 0)]]
            xo_p = [[(pc_sb["xo0"], 0)], [(pc_sb["xo1"], 0)]]

            # ---- per-(parity s, group) tiles; s=0 query-parity, s=1 other
            qt = [persist.tile([128, 512], cdt, tag=f"qt{g}", name=f"qt{g}")
                  for g in range(2)]
            kt = {(sp, g): persist.tile([128, 512], cdt, tag=f"kt{sp}{g}",
                                        name=f"kt{sp}{g}")
                  for sp in range(2) for g in range(2)}
            vt = {(sp, g): persist.tile([128, 512], cdt, tag=f"vt{sp}{g}",
                                        name=f"vt{sp}{g}")
                  for sp in range(2) for g in range(2)}
            v_aug = {}
            for sp in range(2):
                for m in range(SLOTS):
                    t = persist.tile([128, DV + 1], cdt, tag=f"va{sp}{m}",
                                     name=f"va{sp}{m}")
                    nc.vector.memset(t[:, DV:DV + 1], 1.0)
                    v_aug[(sp, m)] = t
            at = {}
            for sp in range(2):
                for m in range(SLOTS):
                    for g in range(2):
                        if m <= 4 * g + 3:
                            at[(sp, m, g)] = persist.tile(
                                [128, 512], cdt, tag=f"at{sp}_{m}_{g}",
                                name=f"at{sp}_{m}_{g}")

            def proj(name, src, dst, gs, copy_eng, pis=None):
                w = w_sb[name]
                for g in gs:
                    pieces = src[g]
                    sel = range(len(pieces)) if pis is None else pis
                    for pi in sel:
                        t, off = pieces[pi]
                        wd = t.shape[-1]
                        ps = mm_ps.tile([128, wd], F32, tag="mm",
                                        name=f"pj{g}_{pi}")
                        for c in range(NCH):
                            nc.tensor.matmul(
                                ps[:],
                                w[:, c, :],
                                t[:, c, :],
                                start=(c == 0),
                                stop=(c == NCH - 1),
                            )
                        copy_eng(dst[g][:, off:off + wd], ps[:])

            # emission in stream-arrival order; each engine's queue
            # executes in order, so never emit ahead of the data
            def vt_blocks(sp, ms):
                for m in ms:
                    vps = mm_ps.tile([128, 128], cdt, tag="mm", name="vps")
                    nc.tensor.transpose(
                        vps[:],
                        vt[(sp, m // 4)][:, (m % 4) * 128:(m % 4 + 1) * 128],
                        ident[:],
                    )
                    dst = v_aug[(sp, m)][:, 0:DV]
                    nc.vector.tensor_copy(dst, vps[:])

            def scores(sp, mgs):
                # S^T for key-slot m of parity sp over q-group g
                for m, g in mgs:
                    lo = max(m, 4 * g)
                    if lo > 4 * g + 3:
                        continue
                    a = lo - 4 * g
                    st = mm_ps.tile([128, 512], F32, tag="mm")
                    nc.tensor.matmul(
                        st[:, a * 128:512],
                        kt[(sp, m // 4)][:, (m % 4) * 128:(m % 4 + 1) * 128],
                        qt[g][:, a * 128:512],
                        start=True, stop=True,
                        skip_group_check=True,
                    )
                    nc.scalar.activation(
                        at[(sp, m, g)][:, a * 128:512],
                        st[:, a * 128:512],
                        mybir.ActivationFunctionType.Exp,
                        bias=0.0, scale=1.0,
                    )
                    if 4 * g <= m <= 4 * g + 3:
                        # causal boundary applied POST-exp as a 0/1
                        # multiply (GpSimd can't touch PSUM): q-parity
                        # slot m gets the triangle, other-parity slot m is
                        # all-or-nothing by core parity (mask data).
                        # |score| <= ~12 so unmasked exp cannot overflow.
                        qoff = (m - 4 * g) * 128
                        asl = at[(sp, m, g)][:, qoff:qoff + 128]
                        nc.gpsimd.tensor_mul(
                            asl, asl, mask_sb[:, sp * 128:(sp + 1) * 128],
                        )

            def av(ks):
                for k in ks:
                    g, q = k // 4, (k % 4) * 128
                    zp = z_ps.tile([128, DV + 1], F32, tag="z")
                    for m in range(k + 1):
                        for sp in range(2):
                            nc.tensor.matmul(
                                zp[:],
                                at[(sp, m, g)][:, q:q + 128],
                                v_aug[(sp, m)][:],
                                start=(m == 0 and sp == 0),
                                stop=(m == k and sp == 1),
                            )
                    rcp = work.tile([128, 1], F32, tag="rcp")
                    nc.vector.reciprocal(rcp[:], zp[:, DV:DV + 1])
                    z_sb = work.tile([128, DV], F32, tag="zout")
                    nc.vector.tensor_scalar_mul(z_sb[:], zp[:, 0:DV], rcp[:])
                    nc.sync.dma_start(
                        out=out_ext[k * 128:(k + 1) * 128, :], in_=z_sb[:]
                    )

            vcopy = nc.vector.tensor_copy
            scopy = nc.scalar.copy
            kt0 = [kt[(0, 0)], kt[(0, 1)]]
            kt1 = [kt[(1, 0)], kt[(1, 1)]]
            vt0 = [vt[(0, 0)], vt[(0, 1)]]
            vt1 = [vt[(1, 0)], vt[(1, 1)]]

            # -- group 0 pieces arrive interleaved with wq/wk/wv; emit
            # Q/K/V per piece in ring-arrival order, with warm-up matmuls
            # filling the early DMA-wait bubbles --
            for pi in range(3):
                proj("wq", xq_p, qt, [0], scopy, pis=[pi])
                proj("wk", xq_p, kt0, [0], scopy, pis=[pi])
                proj("wv", xq_p, vt0, [0], vcopy, pis=[pi])
                vt_blocks(0, [[0], [1], [2, 3]][pi])
                warm(2)
            scores(0, [(m, 0) for m in range(0, 4)])
            # -- xq3 (parity-0 key blocks 4..7) --
            proj("wq", xq_p, qt, [1], scopy)
            proj("wk", xq_p, kt0, [1], scopy)
            proj("wv", xq_p, vt0, [1], vcopy)
            scores(0, [(m, 1) for m in range(0, 4)])
            vt_blocks(0, range(4, 8))
            scores(0, [(m, 1) for m in range(4, 8)])
            # -- xo0 (parity-1 key blocks 0..3) --
            proj("wk", xo_p, kt1, [0], scopy)
            proj("wv", xo_p, vt1, [0], vcopy)
            vt_blocks(1, range(0, 4))
            scores(1, [(m, 0) for m in range(0, 4)])
            av(range(0, 4))   # slots 0..3 need only g=0 of both parities
            scores(1, [(m, 1) for m in range(0, 4)])
            # -- xo1 (parity-1 key blocks 4..7) --
            proj("wk", xo_p, kt1, [1], scopy)
            proj("wv", xo_p, vt1, [1], vcopy)
            vt_blocks(1, range(4, 8))
            scores(1, [(m, 1) for m in range(4, 8)])
            av(range(4, 8))

    nc.finalize()
    return nc


_NC = None


def _get_nc():
    global _NC
    if _NC is None:
        _NC = build_nc()
    return _NC


def _make_masks():
    npdt = _np_cdt()
    p = np.arange(128)[:, None]   # key (partition)
    q = np.arange(128)[None, :]   # query (free)
    # multiplicative 0/1 masks applied to exp(scores)
    triT = (p <= q).astype(npdt)
    dead = np.zeros((128, 128), npdt)
    live = np.ones((128, 128), npdt)
    # col block 0: q-parity key-slot m == k (diagonal, both parities);
    # col block 1: other-parity key-slot m == k (all-masked on even cores,
    # all-valid on odd cores)
    mask_even = np.concatenate([triT, dead], axis=1)
    mask_odd = np.concatenate([triT, live], axis=1)
    return mask_even, mask_odd


def kernel(X, W_Q, W_K, W_V):
    X = np.asarray(X, np.float32)
    W_Q = np.asarray(W_Q, np.float32)
    W_K = np.asarray(W_K, np.float32)
    W_V = np.asarray(W_V, np.float32)

    nc = _get_nc()
    npdt = _np_cdt()
    mask_even, mask_odd = _make_masks()

    def warr(W):
        return np.ascontiguousarray(
            W.astype(npdt).reshape(NCH, 128, DK).transpose(1, 0, 2)
            .reshape(128, NCH * DK)
        )

    wq = warr(W_Q * SCALE)   # fold the 1/sqrt(d_k) into W_Q
    wk = warr(W_K)
    wv = warr(W_V)

    in_maps = []
    for c in range(8):
        b, par = c // 2, c % 2
        # [c, p, l] chunks of X^T for this batch, parity-split and
        # slot-ordered along l
        xt = np.ascontiguousarray(X[b].T).astype(npdt).reshape(NCH, 128, L)
        qcols = np.concatenate(
            [np.arange((2 * k + par) * 128, (2 * k + par + 1) * 128)
             for k in range(SLOTS)]
        )
        ocols = np.concatenate(
            [np.arange((2 * k + 1 - par) * 128, (2 * k + 2 - par) * 128)
             for k in range(SLOTS)]
        )
        src = {"q": xt[:, :, qcols], "o": xt[:, :, ocols]}
        m = {"wq": wq, "wk": wk, "wv": wv,
             "maskT": mask_odd if par else mask_even}
        for nm, s, lo, w in PIECES:
            # pack as [p, c*w+l]: one contiguous DRAM run per partition
            m[nm] = np.ascontiguousarray(
                src[s][:, :, lo:lo + w].transpose(1, 0, 2).reshape(128, NCH * w)
            )
        in_maps.append(m)

    res = run_bass_kernel_spmd(nc, in_maps, list(range(8)))

    Z = np.zeros((B, L, DV), np.float32)
    for c in range(8):
        b, par = c // 2, c % 2
        o = res.results[c]["out"]
        for k in range(SLOTS):
            j = 2 * k + par
            Z[b, j * 128:(j + 1) * 128, :] = o[k * 128:(k + 1) * 128, :]
    return Z
